# revision 2
# baseline (speedup 1.0000x reference)
"""Trainium2 Bass kernel: grayscale + 8x8 block 2D-DCT (torch_dct style, norm=None).

Input  x: (8, 3, 32, 256, 256) f32 video batch.
Output:   (8, 32, 1024, 8, 8) f32 per-block DCT coefficients.

Sharding: fully data-parallel, batch element b -> NeuronCore b (8 cores).

Per-core algorithm, processing images in groups of 4 (t-quad):
  1. Load all 3 channels of each image, h-half at a time: SBUF [128, 3*256]
     (one DMA; per-partition 3x 1 KiB chunks).
  2. Grayscale: g = 0.2989 R + 0.587 G + 0.114 B -> [128, 256] x2.
     First multiply on ScalarE (Copy w/ scale), two FMAs on VectorE.
  3. Pass 1 (H-DCT) on TensorE with the *data as lhsT* (stationary):
       yT[w, (hb,k)] = sum_n g[hb*8+n, w] * D[k, n]
     via matmul(out, lhsT=g_chunk, rhs=E), E = I_16 (x) D^T (block-diag
     128x128): the result comes out already transposed. Accumulated into a
     4-image tile yT4[w_half] = [128 (w), 4*256 (t, hb, k)].
  4. Pass 2 (W-DCT), k-sliced so both frequency indices land in the free dim:
     for each w-octet o and k: matmul with
       lhsT = yT4[rows (wb8,m), cols (t, hb) at fixed k]   (M = 128 = (t,hb))
       rhs  = E[o*64:+64, o*64:+64] = I_8 (x) D^T          (N = 64 = (wb8,l))
     writing PSUM [128 (t,hb), 2048 (wb,k,l)] windows -> final output layout.
  5. Copy PSUM->SBUF (ScalarE) per w-half, then store each half with one
     fully contiguous 512 KiB DMA: DRAM [(t,hb) stride 2048 x128, 1024].

Both matmul passes keep the tensor stationary (lhsT = data, rhs = constant
DCT matrix), so no separate PE transposes are needed anywhere.
"""

import os
import sys

import numpy as np

_TRN_REPO = "/opt/trn_rl_repo"
if _TRN_REPO not in sys.path and os.path.isdir(_TRN_REPO):
    sys.path.insert(0, _TRN_REPO)

import concourse.bass as bass  # noqa: E402
import concourse.tile as tile  # noqa: E402
from concourse import bacc, mybir  # noqa: E402
from concourse.bass_utils import run_bass_kernel_spmd  # noqa: E402

F32 = mybir.dt.float32

# Problem constants (hardcoded per harness contract)
B, C, T, H, W = 8, 3, 32, 256, 256
NB = 8  # DCT block size
HB = H // NB  # 32
WB = W // NB  # 32
P = HB * WB  # 1024

# x DRAM element strides (per-core slice [3, 32, 256, 256])
XS_C = T * H * W
XS_T = H * W
XS_H = W

# out DRAM element strides (per-core slice [32, 1024, 8, 8])
OS_T = P * NB * NB  # 65536

_GRAY_W = (0.2989, 0.587, 0.114)


def _dct_matrix() -> np.ndarray:
    n = np.arange(NB)
    D = 2.0 * np.cos(np.pi * (2.0 * n[None, :] + 1.0) * n[:, None] / (2.0 * NB))
    return D.astype(np.float32)  # [k, n]


def _e_matrix() -> np.ndarray:
    # E[(b, n), (b, k)] = D[k, n]; block diagonal I_16 (x) D^T
    return np.kron(np.eye(16, dtype=np.float32), _dct_matrix().T.copy())


def _build_nc(repeat: int = 1, load_split: bool = False) -> bass.Bass:
    nc = bacc.Bacc(
        "TRN2",
        target_bir_lowering=False,
        debug=False,
        enable_asserts=False,
        num_devices=B,
    )
    x_t = nc.dram_tensor("x", [C, T, H, W], F32, kind="ExternalInput")
    e_t = nc.dram_tensor("e", [128, 128], F32, kind="ExternalInput")
    o_t = nc.dram_tensor("out", [T, P, NB, NB], F32, kind="ExternalOutput")

    with tile.TileContext(nc) as tc:
        with (
            tc.tile_pool(name="const", bufs=1) as const_pool,
            tc.tile_pool(name="xin", bufs=10) as xin_pool,
            tc.tile_pool(name="gray", bufs=8) as gray_pool,
            tc.tile_pool(name="yt4", bufs=4) as yt4_pool,
            tc.tile_pool(name="osb", bufs=3) as osb_pool,
            tc.tile_pool(name="ps1", bufs=1, space="PSUM") as ps1_pool,
            tc.tile_pool(name="ps2", bufs=1, space="PSUM") as ps2_pool,
        ):
            e_sb = const_pool.tile([128, 128], F32)
            # SWDGE queue: keeps the HWDGE ring free for the first input loads
            nc.gpsimd.dma_start(out=e_sb[:], in_=e_t[:, :])

            for tq in range(repeat * (T // 4)):
                tq = tq % (T // 4)
                yt4 = [
                    yt4_pool.tile(
                        [128, 4 * 256], F32, name=f"yt4_{wh}", tag=f"yt4_{wh}"
                    )
                    for wh in range(2)
                ]
                ps1 = [
                    ps1_pool.tile(
                        [128, 4 * 256], F32, name=f"ps1_{wh}", tag=f"ps1_{wh}"
                    )
                    for wh in range(2)
                ]

                for t4 in range(4):
                    t = tq * 4 + t4
                    # ---- load + grayscale, one h-half (128 rows) at a time --
                    g_tiles = []
                    for hh in range(2):
                        xin = xin_pool.tile([128, 3 * W], F32)
                        if load_split:
                            # one fully-contiguous 128 KiB DMA per channel
                            for c in range(C):
                                src = bass.AP(
                                    x_t,
                                    c * XS_C + t * XS_T + hh * 128 * XS_H,
                                    [[XS_H, 128], [1, W]],
                                )
                                nc.sync.dma_start(
                                    out=xin[:, c * W : (c + 1) * W], in_=src
                                )
                        else:
                            src = bass.AP(
                                x_t,
                                t * XS_T + hh * 128 * XS_H,
                                [[XS_H, 128], [XS_C, 3], [1, W]],
                            )
                            nc.sync.dma_start(out=xin[:], in_=src)

                        g = gray_pool.tile([128, W], F32)
                        # first channel on ScalarE (ACT): g = R * w_r
                        nc.scalar.activation(
                            g[:], xin[:, 0:W],
                            mybir.ActivationFunctionType.Copy,
                            scale=float(_GRAY_W[0]),
                        )
                        nc.vector.scalar_tensor_tensor(
                            g[:], xin[:, W : 2 * W], _GRAY_W[1], g[:],
                            op0=mybir.AluOpType.mult, op1=mybir.AluOpType.add,
                        )
                        nc.vector.scalar_tensor_tensor(
                            g[:], xin[:, 2 * W : 3 * W], _GRAY_W[2], g[:],
                            op0=mybir.AluOpType.mult, op1=mybir.AluOpType.add,
                        )
                        g_tiles.append(g)

                    # ---- pass 1: H-DCT, transposed out: yT[w, (hb,k)] ----
                    for wh in range(2):
                        for hh in range(2):
                            nc.tensor.matmul(
                                ps1[wh][
                                    :,
                                    t4 * 256 + hh * 128 : t4 * 256 + (hh + 1) * 128,
                                ],
                                lhsT=g_tiles[hh][:, wh * 128 : (wh + 1) * 128],
                                rhs=e_sb[:],
                                start=True,
                                stop=True,
                            )
                        # per-image drain so pass1(g+1) isn't gated on one
                        # big end-of-group copy
                        nc.vector.tensor_copy(
                            yt4[wh][:, t4 * 256 : (t4 + 1) * 256],
                            ps1[wh][:, t4 * 256 : (t4 + 1) * 256],
                        )

                # ---- pass 2: W-DCT, k-sliced; out [(t,hb), (wb,k,l)] ----
                osb = osb_pool.tile([128, 2048], F32)
                for wh in range(2):
                    ps2 = ps2_pool.tile(
                        [128, 1024], F32, name=f"ps2_{wh}", tag=f"ps2_{wh}"
                    )
                    # [64, t, hb, k] per octet
                    yv = yt4[wh][:].rearrange(
                        "p (t hb k) -> p t hb k", t=4, hb=HB, k=NB
                    )
                    pv = ps2[:].rearrange(
                        "p (o wb k l) -> p o wb k l", o=2, wb=8, k=NB, l=NB
                    )
                    for wq in range(2):
                        rhs = e_sb[wq * 64 : (wq + 1) * 64, wq * 64 : (wq + 1) * 64]
                        for k in range(NB):
                            nc.tensor.matmul(
                                pv[:, wq, :, k, :],
                                lhsT=yv[wq * 64 : (wq + 1) * 64, :, :, k],
                                rhs=rhs,
                                start=True,
                                stop=True,
                            )
                    if tq == T // 4 - 1:
                        # final group: drain per w-octet on the idle DVE and
                        # store 256 KiB quarters — shortens the drain tail
                        for wq in range(2):
                            off = wh * 1024 + wq * 512
                            nc.vector.tensor_copy(
                                osb[:, off : off + 512],
                                ps2[:, wq * 512 : (wq + 1) * 512],
                            )
                            dst = bass.AP(
                                o_t,
                                tq * 4 * OS_T + off,
                                [[2048, 128], [1, 512]],
                            )
                            nc.scalar.dma_start(
                                out=dst, in_=osb[:, off : off + 512]
                            )
                    else:
                        nc.scalar.copy(
                            osb[:, wh * 1024 : (wh + 1) * 1024], ps2[:]
                        )
                        dst = bass.AP(
                            o_t,
                            tq * 4 * OS_T + wh * 1024,
                            [[2048, 128], [1, 1024]],
                        )
                        nc.scalar.dma_start(
                            out=dst, in_=osb[:, wh * 1024 : (wh + 1) * 1024]
                        )

    nc.compile()
    return nc


_NC = {}


def _get_nc(repeat: int = 1):
    if repeat not in _NC:
        _NC[repeat] = _build_nc(repeat)
    return _NC[repeat]


def _in_maps(x: np.ndarray):
    x = np.ascontiguousarray(np.asarray(x), dtype=np.float32)
    assert x.shape == (B, C, T, H, W), x.shape
    e = _e_matrix()
    return [{"x": x[i], "e": e} for i in range(B)]


def _run(x: np.ndarray, repeat: int = 1, **kwargs):
    in_maps = _in_maps(x)
    res = run_bass_kernel_spmd(_get_nc(repeat), in_maps, list(range(B)), **kwargs)
    out = np.stack([res.results[i]["out"] for i in range(B)], axis=0)
    return out, res


def kernel(x: np.ndarray) -> np.ndarray:
    out, _ = _run(x)
    return out

